# revision 20
# baseline (speedup 1.0000x reference)
# Trainium2 Bass kernel for nn_MultiHeadAttention (B=2, S=2048, D=1024, H=16).
#
# Sharding: head-tensor-parallel over 8 cores. Core c computes heads
# {2c, 2c+1}: column-sharded wq/wk/wv (128 output dims per core),
# row-sharded wo with the partial-output sum done on the host.
#
# Device layout strategy: all activations live transposed (feature-dim on
# partitions) so the PE never needs activation transposes:
#   QT/KT = (128 e_local, 4096 bs) computed with weight tiles stationary.
#   Scores are computed in transposed layout scoresT = (k, q): the softmax
#   exp then runs PSUM->SBUF on the ACT engine and directly yields P^T in
#   the layout the AV matmul needs (k on partitions). Row-sums come from an
#   extra ones-column appended to V (matmul computes them for free).
#   Mask bias is added in-PSUM via an identity-weight matmul.
# All matmuls use float32r (full-rate fp32 PE mode; fp32 proper is 4x slower).

import numpy as np

B, S, D, H = 2, 2048, 1024, 16
DK = D // H            # 64
NC = 8                 # cores
EL = D // NC           # 128 local e-dims (2 heads)
BS = B * S             # 4096 flattened tokens
NCH = 8                # projection bs-chunks of 512
CH = BS // NCH         # 512
NDT = D // 128         # 8 contraction tiles
NKT = S // 128         # 16 k-tiles per batch
NQB = S // 512         # 4 q-blocks per batch
NEG = np.float32(-1.0e30)

SKIP, PLAIN = -1, -2   # block classes (>=0 means bias tile index)


def _classify_mask(mask):
    """Per (kt, qj) block classification of the (S_q, S_k) mask.

    Returns (cls[kt][qj], bias_blocks (U,128,512) in transposed (k,q)
    orientation)."""
    m = np.asarray(mask).reshape(S, S)          # [q, k]; 0 = masked
    biasT = np.where(m == 0, NEG, np.float32(0)).T  # [k, q]
    biasT = np.ascontiguousarray(biasT)
    cls = [[PLAIN] * NQB for _ in range(NKT)]
    uniq = {}
    blocks = []
    for kt in range(NKT):
        for qj in range(NQB):
            blk = biasT[kt * 128:(kt + 1) * 128, qj * 512:(qj + 1) * 512]
            if not (blk != 0).any():
                cls[kt][qj] = PLAIN
            elif (blk != 0).all():
                cls[kt][qj] = SKIP
            else:
                key = blk.tobytes()
                if key not in uniq:
                    uniq[key] = len(blocks)
                    blocks.append(np.ascontiguousarray(blk))
                cls[kt][qj] = uniq[key]
    if blocks:
        mb = np.stack(blocks)
    else:
        mb = np.zeros((0, 128, 512), np.float32)
    return cls, mb


def _build_program(cls, n_bias):
    import concourse.bacc as bacc
    import concourse.mybir as mybir
    from concourse.tile import TileContext

    f32 = mybir.dt.float32
    f32r = mybir.dt.float32r
    f16 = mybir.dt.float16
    Exp = mybir.ActivationFunctionType.Exp
    mult = mybir.AluOpType.mult

    nc = bacc.Bacc("TRN2", target_bir_lowering=False, debug=False,
                   num_devices=NC)

    qT = nc.dram_tensor("qT", [D, BS], f16, kind="ExternalInput")
    kT = nc.dram_tensor("kT", [D, BS], f16, kind="ExternalInput")
    vT = nc.dram_tensor("vT", [D, BS], f16, kind="ExternalInput")
    w3d = nc.dram_tensor("w3", [128, 3 * NDT * EL], f16,
                         kind="ExternalInput")
    woT = nc.dram_tensor("woT", [EL, D], f16, kind="ExternalInput")
    bias3 = nc.dram_tensor("bias3", [EL, 3], f32, kind="ExternalInput")
    # cst = [ident(128) | n_bias x maskbias(512)]
    cstd = nc.dram_tensor("cst", [128, 128 + n_bias * 512], f32r,
                          kind="ExternalInput")
    out = nc.dram_tensor("out", [BS, D], f16, kind="ExternalOutput")

    # transposed-input views: [p, t, c] with t the 128-row block
    qT_r = qT.ap().rearrange("(t p) c -> p t c", p=128)
    kT_r = kT.ap().rearrange("(t p) c -> p t c", p=128)
    vT_r = vT.ap().rearrange("(t p) c -> p t c", p=128)

    with TileContext(nc) as tc:
        with (
            tc.tile_pool(name="const", bufs=1) as constp,
            tc.tile_pool(name="per", bufs=1) as perp,
            tc.tile_pool(name="stage", bufs=3) as stagep,
            tc.tile_pool(name="vt", bufs=3) as vtp,
            tc.tile_pool(name="pt", bufs=3) as ptp,
            tc.tile_pool(name="zz", bufs=3) as zzp,
            tc.tile_pool(name="zb", bufs=4) as zbp,
            tc.tile_pool(name="ost", bufs=3) as ostp,
            tc.tile_pool(name="psA", bufs=2, space="PSUM") as psA,
            tc.tile_pool(name="psS", bufs=2, space="PSUM") as psS,
            tc.tile_pool(name="psO", bufs=2, space="PSUM") as psO,
        ):
            # ---- constants (packed; w3 first so projections start asap,
            # the rest are emitted after the first two input chunks) ----
            w3 = constp.tile([128, 3, NDT, EL], f16, tag="w3")
            nc.sync.dma_start(
                out=w3[:],
                in_=w3d.ap().rearrange("p (j t e) -> p j t e", j=3, t=NDT))
            w_sb = {"q": w3[:, 0], "k": w3[:, 1], "v": w3[:, 2]}
            b3 = constp.tile([EL, 3], f32, tag="b3")
            nc.sync.dma_start(out=b3[:], in_=bias3.ap()[:])
            cst = constp.tile([128, 128 + n_bias * 512], f32r, tag="cst")
            ident = cst[:, 0:128]
            mb_sb = [cst[:, 128 + u * 512:128 + (u + 1) * 512]
                     for u in range(n_bias)]
            woT_sb = constp.tile([EL, D], f16, tag="wo")

            def emit_rest_consts():
                nc.sync.dma_start(out=cst[:], in_=cstd.ap()[:])
                nc.sync.dma_start(out=woT_sb[:], in_=woT.ap()[:])

            # ---- persistent activations ----
            QT_sb = perp.tile([EL, BS], f32r, tag="QT")
            KT_sb = perp.tile([EL, BS], f32r, tag="KT")
            OT_sb = perp.tile([EL, BS], f16, tag="OT")
            # V in natural (bs, e) layout, packed per 128-row tile g as
            # 4 chunks of 64 cols: [V_A | ones | V_B | ones] so each head's
            # stationary operand [V_h | onescol] is one contiguous 128-col AP
            V_big = perp.tile([128, 4 * (BS // 128) * 64], f32r, tag="Vb")
            V3 = V_big[:].rearrange("p (t x) -> p t x", x=64)

            def emit_vz():
                # memset cannot target f32r: build the [1,0,...,0] ones-chunk
                # pattern in an f16 staging tile and cast-copy it in
                vz = stagep.tile([128, (BS // 128) * 2 * 64], f16,
                                 tag="stage")
                vz3 = vz[:].rearrange("p (t x) -> p t x", x=64)
                nc.vector.memset(vz[:], 0.0)
                nc.vector.memset(vz3[:, :, 0:1], 1.0)
                nc.vector.tensor_copy(V3[:, 1::2, :], vz3[:])

            # ---- projections, batch-major so batch-0 attention can start
            # while batch-1 inputs are still streaming ----
            def emit_proj_chunk(j, name, src_r, dst, c):
                w = w_sb[name]  # AP view [p, t, e]
                st = stagep.tile([128, NDT, CH], f16, tag="stage")
                nc.sync.dma_start(out=st[:],
                                  in_=src_r[:, :, c * CH:(c + 1) * CH])
                ps = psA.tile([EL, CH], f32, tag="proj")
                for t in range(NDT):
                    nc.tensor.matmul(ps[:], w[:, t, :], st[:, t, :],
                                     start=(t == 0), stop=(t == NDT - 1))
                if dst is not None:
                    nc.vector.tensor_scalar_add(
                        dst[:, c * CH:(c + 1) * CH], ps[:], b3[:, j:j + 1])
                else:
                    vt = vtp.tile([EL, CH], f32r, tag="vt")
                    nc.vector.tensor_scalar_add(vt[:], ps[:], b3[:, j:j + 1])
                    # transpose this VT chunk into V_big right away
                    for gg in range(CH // 128):
                        g = c * (CH // 128) + gg
                        tp = psA.tile([128, 128], f32r, tag="proj")
                        nc.tensor.transpose(
                            tp[:], vt[:, gg * 128:(gg + 1) * 128], ident)
                        nc.vector.tensor_copy(
                            V3[:, 4 * g:4 * g + 3:2, :],
                            tp[:].rearrange("p (a b) -> p a b", b=64))

            JT = (("q", qT_r, QT_sb), ("k", kT_r, KT_sb), ("v", vT_r, None))

            def emit_projections(b, skip=0):
                n = 0
                for c in range(b * (NCH // B), (b + 1) * (NCH // B)):
                    for j, (name, src_r, dst) in enumerate(JT):
                        n += 1
                        if n > skip:
                            emit_proj_chunk(j, name, src_r, dst, c)

            # ---- attention ----
            def emit_attention(b):
                for qj in range(NQB):
                    qlo = b * S + qj * 512
                    for h in range(2):
                        hs = slice(h * 64, (h + 1) * 64)
                        acts = [kt for kt in range(NKT)
                                if cls[kt][qj] != SKIP]
                        if not acts:
                            continue
                        ot = psO.tile([128, 512], f32, tag="ot")
                        n_done = 0
                        for p0 in range(0, NKT, 2):
                            pair = [kt for kt in (p0, p0 + 1) if kt in acts]
                            if not pair:
                                continue
                            sc = psS.tile([128, 1024], f32, tag="score")
                            for kt in pair:
                                i = kt - p0
                                half = sc[:, i * 512:(i + 1) * 512]
                                klo = b * S + kt * 128
                                cl = cls[kt][qj]
                                nc.tensor.matmul(
                                    half,
                                    KT_sb[hs, klo:klo + 128],
                                    QT_sb[hs, qlo:qlo + 512],
                                    start=True, stop=(cl == PLAIN))
                                if cl >= 0:
                                    nc.tensor.matmul(
                                        half, ident,
                                        mb_sb[cl],
                                        start=False, stop=True)
                            pt = ptp.tile([128, 1024], f32r, tag="pt")
                            lo = (pair[0] - p0) * 512
                            hi = (pair[-1] - p0 + 1) * 512
                            nc.scalar.activation(pt[:, lo:hi], sc[:, lo:hi],
                                                 Exp, scale=0.125)
                            for kt in pair:
                                i = kt - p0
                                g = b * NKT + kt
                                vap = V_big[:, g * 256 + h * 128:
                                            g * 256 + (h + 1) * 128]
                                n_done += 1
                                nc.tensor.matmul(
                                    ot[:], vap,
                                    pt[:, i * 512:(i + 1) * 512],
                                    start=(n_done == 1),
                                    stop=(n_done == len(acts)))
                        # normalize: row 64 of ot = Z (sum of exp)
                        z = zzp.tile([1, 512], f32, tag="z")
                        nc.vector.tensor_copy(z[:], ot[64:65, :])
                        zb = zbp.tile([64, 512], f32, tag="zb")
                        nc.gpsimd.partition_broadcast(zb[:], z[:],
                                                      channels=64)
                        rb = zbp.tile([64, 512], f32, tag="zb")
                        nc.vector.reciprocal_approx_fast(rb[:], zb[:])
                        nc.vector.tensor_tensor(
                            OT_sb[hs, qlo:qlo + 512], ot[0:64, :], rb[:],
                            op=mult)
                    emit_oproj_qblock(b, qj)

            # ---- output projection (partial over local e-dims),
            # per q-block so outputs stream during attention ----
            def emit_oproj_qblock(b, qj):
                for g in range(b * 16 + qj * 4, b * 16 + (qj + 1) * 4):
                    osr = ostp.tile([128, D], f16, tag="ost")
                    for j in range(2):
                        po = psO.tile([128, 512], f32, tag="ot")
                        nc.tensor.matmul(po[:],
                                         OT_sb[:, g * 128:(g + 1) * 128],
                                         woT_sb[:, j * 512:(j + 1) * 512],
                                         start=True, stop=True)
                        if j == 0:
                            nc.vector.tensor_copy(
                                osr[:, j * 512:(j + 1) * 512], po[:])
                        else:
                            nc.scalar.copy(
                                osr[:, j * 512:(j + 1) * 512], po[:])
                    nc.sync.dma_start(out=out.ap()[g * 128:(g + 1) * 128, :],
                                      in_=osr[:])

            for b in range(B):
                if b == 0:
                    # first q,k chunks stream before the remaining consts
                    emit_proj_chunk(0, *JT[0], 0)
                    emit_proj_chunk(1, *JT[1], 0)
                    emit_rest_consts()
                    emit_vz()
                    emit_projections(0, skip=2)
                else:
                    emit_projections(b)
                emit_attention(b)

    nc.compile()
    return nc


_CACHE = {}


def kernel(q, k, v, mask, wq, bq, wk, bk, wv, bv, wo, bo):
    from concourse.bass_utils import run_bass_kernel_spmd

    q = np.ascontiguousarray(np.asarray(q, np.float32).reshape(BS, D))
    k = np.ascontiguousarray(np.asarray(k, np.float32).reshape(BS, D))
    v = np.ascontiguousarray(np.asarray(v, np.float32).reshape(BS, D))
    wq = np.asarray(wq, np.float32)
    wk = np.asarray(wk, np.float32)
    wv = np.asarray(wv, np.float32)
    wo = np.asarray(wo, np.float32)
    bq = np.asarray(bq, np.float32)
    bk = np.asarray(bk, np.float32)
    bv = np.asarray(bv, np.float32)
    bo = np.asarray(bo, np.float32)

    qTf = np.ascontiguousarray(q.T.astype(np.float16))
    kTf = np.ascontiguousarray(k.T.astype(np.float16))
    vTf = np.ascontiguousarray(v.T.astype(np.float16))

    cls, mb = _classify_mask(mask)
    key = (tuple(tuple(r) for r in cls), len(mb))
    if key not in _CACHE:
        _CACHE[key] = _build_program(cls, len(mb))
    nc = _CACHE[key]

    # cst = [ident | mask-bias blocks], one DMA on device
    cst = np.concatenate([np.eye(128, dtype=np.float32)]
                         + [mb[u] for u in range(len(mb))], axis=1)
    cst = np.ascontiguousarray(cst)

    def pack_w3(c):
        el = slice(c * EL, (c + 1) * EL)
        ws = []
        for w in (wq, wk, wv):
            wt = np.ascontiguousarray(w[el, :].T.astype(np.float16))
            ws.append(wt.reshape(NDT, 128, EL).transpose(1, 0, 2))
        return np.ascontiguousarray(
            np.stack(ws, axis=1).reshape(128, 3 * NDT * EL))

    in_maps = []
    for c in range(NC):
        el = slice(c * EL, (c + 1) * EL)
        m = {
            "qT": qTf, "kT": kTf, "vT": vTf,
            "w3": pack_w3(c),
            "woT": np.ascontiguousarray(wo[:, el].T.astype(np.float16)),
            "bias3": np.ascontiguousarray(
                np.stack([bq[el], bk[el], bv[el]], axis=1)),
            "cst": cst,
        }
        in_maps.append(m)

    res = run_bass_kernel_spmd(nc, in_maps, list(range(NC)))
    acc = res.results[0]["out"].astype(np.float32)
    for c in range(1, NC):
        acc = acc + res.results[c]["out"]
    acc = acc + bo[None, :]
    return acc.reshape(B, S, D)
